# revision 1
# baseline (speedup 1.0000x reference)
"""Multi-head attention (B=4, S=2048, D=1024, H=16, E=64) on 8 TRN2 NeuronCores.

Sharding: core c handles batch b=c//2 and query-half qh=c%2 (1024 query tokens).
K/V are computed per-core for the full 2048-token sequence of its batch (2x
duplicated K/V projection work, but zero collectives / zero cross-core deps).

Per-core program (SPMD, identical on all cores):
  phase 0: V = x @ wv + bv for all 16 heads, stored [tok, head, 65] with a
           ones-column appended per head (gives softmax sums for free during
           att@V), spilled to DRAM scratch.
  passes p=0..7 (heads 2p, 2p+1):
    KT[128he, 2048tok] = (wk_p.T @ xT) + bk  (transposed layout)
    QT[128he, 1024tq]  = (wq_p.T @ xTq) + bq
    per (head, tq-tile of 512):
      scoresT[tk,tq] = KT_h.T-slices @ QT_h  (K=64 matmuls, one per tk-tile)
      exp on ScalarE straight from PSUM with scale=1/8 (softmax max-subtraction
      skipped: |score/8| <= ~12 so exp is fp32-safe)
      attT[65,tq] += [V_h | 1].T @ expT  (row 64 accumulates the softmax sum)
      normalize: recip(sum) broadcast via a K=1 matmul, multiply on VectorE
  phase 2: out[tok,1024] = [att | 1] @ [wo.T ; bo]  (bias via ones-row matmul)

All matmuls run in float32r (tf32-like, full PE rate at N>=512).
"""

import numpy as np

import concourse.bass as bass
import concourse.mybir as mybir
import concourse.tile as tile
from concourse import bacc
from concourse.bass_utils import run_bass_kernel_spmd

FP32 = mybir.dt.float32
FP32R = mybir.dt.float32r
AF = mybir.ActivationFunctionType

B, S, D, H, E = 4, 2048, 1024, 16, 64
NCORES = 8
TQ = S // 2  # query tokens per core
SCALE = 1.0 / float(np.sqrt(E))

_CACHE = {}


def build_nc():
    nc = bacc.Bacc("TRN2", target_bir_lowering=False)

    xT = nc.dram_tensor("xT", [D, S], FP32R, kind="ExternalInput")
    xTq = nc.dram_tensor("xTq", [D, TQ], FP32R, kind="ExternalInput")
    wq_t = nc.dram_tensor("wq_t", [D, H * E], FP32R, kind="ExternalInput")
    wk_t = nc.dram_tensor("wk_t", [D, H * E], FP32R, kind="ExternalInput")
    wv_t = nc.dram_tensor("wv_t", [D, H * E], FP32R, kind="ExternalInput")
    wo_t = nc.dram_tensor("wo_t", [D, D], FP32R, kind="ExternalInput")
    bqp = nc.dram_tensor("bqp", [128, 8], FP32, kind="ExternalInput")
    bkp = nc.dram_tensor("bkp", [128, 8], FP32, kind="ExternalInput")
    bv_row = nc.dram_tensor("bv_row", [1, H * E], FP32R, kind="ExternalInput")
    bo_row = nc.dram_tensor("bo_row", [1, D], FP32R, kind="ExternalInput")
    out = nc.dram_tensor("out", [TQ, D], FP32, kind="ExternalOutput")
    # V spill: [tok-tile, tok-in-tile, head, E+1]
    v_spill = nc.dram_tensor("v_spill", [16, 128, H, E + 1], FP32R)

    xT_r = xT.rearrange("(t p) s -> p t s", p=128)  # [128, 8, 2048]
    xTq_r = xTq.rearrange("(t p) s -> p t s", p=128)  # [128, 8, 1024]
    wq_r = wq_t.rearrange("(t p) m -> p t m", p=128)  # [128, 8, 1024]
    wk_r = wk_t.rearrange("(t p) m -> p t m", p=128)
    wv_r = wv_t.rearrange("(t p) m -> p t m", p=128)
    wo_r = wo_t.rearrange("(t p) m -> p t m", p=128)

    with tile.TileContext(nc) as tc:
        with (
            tc.tile_pool(name="xt", bufs=1) as xt_pool,
            tc.tile_pool(name="wkq", bufs=1) as wkq_pool,
            tc.tile_pool(name="bigw", bufs=2) as bigw_pool,
            tc.tile_pool(name="qt", bufs=2) as qt_pool,
            tc.tile_pool(name="vbuf", bufs=2) as vbuf_pool,
            tc.tile_pool(name="expp", bufs=2) as exp_pool,
            tc.tile_pool(name="attT", bufs=8) as attT_pool,
            tc.tile_pool(name="small", bufs=2) as small_pool,
            tc.tile_pool(name="ones", bufs=1) as ones_pool,
            tc.tile_pool(name="ps_s", bufs=2, space="PSUM") as ps_scores,
            tc.tile_pool(name="ps_a", bufs=2, space="PSUM") as ps_att,
            tc.tile_pool(name="ps_g", bufs=2, space="PSUM") as ps_gen,
        ):
            # ---- persistent tiles ----
            xt_sb = xt_pool.tile([128, 8, S], FP32R, tag="xt")  # 64KB/part
            xtq_sb = xt_pool.tile([128, 8, TQ], FP32R, tag="xtq")  # 32KB/part
            nc.sync.dma_start(out=xt_sb, in_=xT_r)
            nc.sync.dma_start(out=xtq_sb, in_=xTq_r)

            ones_row_f = ones_pool.tile([1, 128], FP32, tag="onesrf")
            nc.vector.memset(ones_row_f, 1.0)
            ones_sb = ones_pool.tile([1, 128], FP32R, tag="ones")
            nc.vector.tensor_copy(out=ones_sb, in_=ones_row_f)
            ones_col_f = ones_pool.tile([128, 8], FP32, tag="onescf")
            nc.vector.memset(ones_col_f, 1.0)
            ones_col = ones_pool.tile([128, 8], FP32R, tag="onescol")
            nc.vector.tensor_copy(out=ones_col, in_=ones_col_f)
            bq_sb = ones_pool.tile([128, 8], FP32, tag="bq")
            bk_sb = ones_pool.tile([128, 8], FP32, tag="bk")
            nc.sync.dma_start(out=bq_sb, in_=bqp[:, :])
            nc.sync.dma_start(out=bk_sb, in_=bkp[:, :])
            bv_sb = ones_pool.tile([1, H * E], FP32R, tag="bv")
            bo_sb = ones_pool.tile([1, D], FP32R, tag="bo")
            nc.sync.dma_start(out=bv_sb, in_=bv_row[:, :])
            nc.sync.dma_start(out=bo_sb, in_=bo_row[:, :])

            attT_tiles = [
                attT_pool.tile([128, TQ], FP32R, tag="attT", name=f"attT{i}")
                for i in range(8)
            ]

            # ---- phase 0: V projection for all heads, spill to DRAM ----
            for nt in range(2):
                wv_sb = bigw_pool.tile([128, 8, 512], FP32R, tag="bigw")
                nc.sync.dma_start(out=wv_sb, in_=wv_r[:, :, nt * 512 : (nt + 1) * 512])
                for tokt in range(16):
                    ps = ps_gen.tile([128, 512], FP32, tag="gen")
                    for k in range(8):
                        nc.tensor.matmul(
                            out=ps,
                            lhsT=xt_sb[:, k, tokt * 128 : (tokt + 1) * 128],
                            rhs=wv_sb[:, k, :],
                            start=(k == 0),
                            stop=False,
                        )
                    nc.tensor.matmul(
                        out=ps,
                        lhsT=ones_sb[:, :128],
                        rhs=bv_sb[:, nt * 512 : (nt + 1) * 512],
                        start=False,
                        stop=True,
                    )
                    vstage = vbuf_pool.tile([128, 8, E + 1], FP32R, tag="vbuf")
                    nc.vector.tensor_copy(
                        out=vstage[:, :, :E],
                        in_=ps.rearrange("p (h e) -> p h e", e=E),
                    )
                    nc.vector.tensor_copy(
                        out=vstage[:, :, E : E + 1], in_=ones_col.unsqueeze(2)
                    )
                    nc.sync.dma_start(
                        out=v_spill[tokt, :, nt * 8 : (nt + 1) * 8, :], in_=vstage
                    )

            # ---- passes: 2 heads each ----
            for p in range(8):
                wk_sb = wkq_pool.tile([128, 8, 128], FP32R, tag="wk")
                wq_sb = wkq_pool.tile([128, 8, 128], FP32R, tag="wq")
                nc.sync.dma_start(out=wk_sb, in_=wk_r[:, :, p * 128 : (p + 1) * 128])
                nc.sync.dma_start(out=wq_sb, in_=wq_r[:, :, p * 128 : (p + 1) * 128])

                kt_sb = bigw_pool.tile([128, S], FP32R, tag="bigw")
                qt_sb = qt_pool.tile([128, TQ], FP32R, tag="qt")

                for ts in range(4):
                    ps = ps_gen.tile([128, 512], FP32, tag="gen")
                    for k in range(8):
                        nc.tensor.matmul(
                            out=ps,
                            lhsT=wk_sb[:, k, :],
                            rhs=xt_sb[:, k, ts * 512 : (ts + 1) * 512],
                            start=(k == 0),
                            stop=(k == 7),
                        )
                    nc.vector.tensor_scalar_add(
                        out=kt_sb[:, ts * 512 : (ts + 1) * 512],
                        in0=ps,
                        scalar1=bk_sb[:, p : p + 1],
                    )
                for qs in range(2):
                    ps = ps_gen.tile([128, 512], FP32, tag="gen")
                    for k in range(8):
                        nc.tensor.matmul(
                            out=ps,
                            lhsT=wq_sb[:, k, :],
                            rhs=xtq_sb[:, k, qs * 512 : (qs + 1) * 512],
                            start=(k == 0),
                            stop=(k == 7),
                        )
                    nc.vector.tensor_scalar_add(
                        out=qt_sb[:, qs * 512 : (qs + 1) * 512],
                        in0=ps,
                        scalar1=bq_sb[:, p : p + 1],
                    )

                for hh in range(2):
                    base = hh * 64
                    h = 2 * p + hh
                    vh_sb = vbuf_pool.tile([128, 16, E + 1], FP32R, tag="vbuf")
                    nc.sync.dma_start(
                        out=vh_sb, in_=v_spill[:, :, h, :].transpose([1, 0, 2])
                    )
                    for tqt in range(2):
                        att_ps = ps_att.tile([E + 1, 512], FP32, tag="att")
                        for g in range(8):
                            ps_s = ps_scores.tile([128, 2, 512], FP32, tag="sc")
                            for j in range(2):
                                t = g * 2 + j
                                nc.tensor.matmul(
                                    out=ps_s[:, j, :],
                                    lhsT=kt_sb[
                                        base : base + 64, t * 128 : (t + 1) * 128
                                    ],
                                    rhs=qt_sb[
                                        base : base + 64, tqt * 512 : (tqt + 1) * 512
                                    ],
                                    start=True,
                                    stop=True,
                                )
                            exp_t = exp_pool.tile([128, 2, 512], FP32R, tag="exp")
                            nc.scalar.activation(
                                out=exp_t, in_=ps_s, func=AF.Exp, scale=SCALE
                            )
                            for j in range(2):
                                t = g * 2 + j
                                nc.tensor.matmul(
                                    out=att_ps,
                                    lhsT=vh_sb[:, t, :],
                                    rhs=exp_t[:, j, :],
                                    start=(t == 0),
                                    stop=(t == 15),
                                )
                        recip_r = small_pool.tile([1, 512], FP32R, tag="recr", bufs=1)
                        with nc.allow_low_precision(reason="fp32r recip for softmax"):
                            nc.vector.reciprocal(out=recip_r, in_=att_ps[E : E + 1, :])
                        rb_ps = ps_gen.tile([64, 512], FP32, tag="gen")
                        nc.tensor.matmul(
                            out=rb_ps,
                            lhsT=ones_sb[:, :64],
                            rhs=recip_r,
                            start=True,
                            stop=True,
                        )
                        rb_sb = small_pool.tile([64, 512], FP32, tag="stg", bufs=2)
                        nc.vector.tensor_copy(out=rb_sb, in_=rb_ps)
                        nc.vector.tensor_mul(
                            out=attT_tiles[p][
                                base : base + 64, tqt * 512 : (tqt + 1) * 512
                            ],
                            in0=att_ps[:E, :],
                            in1=rb_sb,
                        )

            # ---- phase 2: output projection ----
            wo_sb = [
                bigw_pool.tile([128, 8, 512], FP32R, tag="bigw", name=f"wo{i}")
                for i in range(2)
            ]
            for nt in range(2):
                nc.sync.dma_start(
                    out=wo_sb[nt], in_=wo_r[:, :, nt * 512 : (nt + 1) * 512]
                )
            for tokt in range(8):
                for nt in range(2):
                    ps = ps_gen.tile([128, 512], FP32, tag="gen")
                    for t in range(8):
                        nc.tensor.matmul(
                            out=ps,
                            lhsT=attT_tiles[t][:, tokt * 128 : (tokt + 1) * 128],
                            rhs=wo_sb[nt][:, t, :],
                            start=(t == 0),
                            stop=False,
                        )
                    nc.tensor.matmul(
                        out=ps,
                        lhsT=ones_sb[:, :128],
                        rhs=bo_sb[:, nt * 512 : (nt + 1) * 512],
                        start=False,
                        stop=True,
                    )
                    ostg = small_pool.tile([128, 512], FP32, tag="stg", bufs=2)
                    nc.vector.tensor_copy(out=ostg, in_=ps)
                    nc.sync.dma_start(
                        out=out[tokt * 128 : (tokt + 1) * 128, nt * 512 : (nt + 1) * 512],
                        in_=ostg,
                    )

    nc.compile()
    return nc


def kernel(x, wq, bq, wk, bk, wv, bv, wo, bo, trace=False):
    x = np.asarray(x, dtype=np.float32)
    wq = np.asarray(wq, dtype=np.float32)
    bq = np.asarray(bq, dtype=np.float32)
    wk = np.asarray(wk, dtype=np.float32)
    bk = np.asarray(bk, dtype=np.float32)
    wv = np.asarray(wv, dtype=np.float32)
    bv = np.asarray(bv, dtype=np.float32)
    wo = np.asarray(wo, dtype=np.float32)
    bo = np.asarray(bo, dtype=np.float32)

    if "nc" not in _CACHE:
        _CACHE["nc"] = build_nc()
    nc = _CACHE["nc"]

    wq_t = np.ascontiguousarray(wq.transpose(1, 0, 2).reshape(D, H * E))
    wk_t = np.ascontiguousarray(wk.transpose(1, 0, 2).reshape(D, H * E))
    wv_t = np.ascontiguousarray(wv.transpose(1, 0, 2).reshape(D, H * E))
    wo_t = np.ascontiguousarray(wo.T)
    bqp = np.ascontiguousarray(bq.reshape(H * E).reshape(8, 128).T)
    bkp = np.ascontiguousarray(bk.reshape(H * E).reshape(8, 128).T)
    bv_row = np.ascontiguousarray(bv.reshape(1, H * E))
    bo_row = np.ascontiguousarray(bo.reshape(1, D))

    shared = {
        "wq_t": wq_t,
        "wk_t": wk_t,
        "wv_t": wv_t,
        "wo_t": wo_t,
        "bqp": bqp,
        "bkp": bkp,
        "bv_row": bv_row,
        "bo_row": bo_row,
    }
    in_maps = []
    for c in range(NCORES):
        b, qh = c // 2, c % 2
        xT_c = np.ascontiguousarray(x[b].T)
        m = dict(shared)
        m["xT"] = xT_c
        m["xTq"] = np.ascontiguousarray(xT_c[:, qh * TQ : (qh + 1) * TQ])
        in_maps.append(m)

    res = run_bass_kernel_spmd(nc, in_maps, list(range(NCORES)), trace=trace)

    out = np.empty((B, S, D), dtype=np.float32)
    for c in range(NCORES):
        b, qh = c // 2, c % 2
        out[b, qh * TQ : (qh + 1) * TQ, :] = res.results[c]["out"]
    if trace:
        return out, res
    return out



# revision 2
# speedup vs baseline: 1.3598x; 1.3598x over previous
"""Multi-head attention (B=4, S=2048, D=1024, H=16, E=64) on 8 TRN2 NeuronCores.

Sharding: core c handles batch b=c//2 and query-half qh=c%2 (1024 query tokens).
K/V are computed per-core for the full 2048-token sequence of its batch (2x
duplicated K/V projection work, but zero collectives / zero cross-core deps).

The host reorders each core's xT so the core's OWN query tokens occupy columns
0..1023 (attention is invariant to key/value token order as long as K and V
share it), so a single SPMD program serves all cores without a separate xTq
input.

Per-core program (SPMD, identical on all cores):
  phase 0: V = x @ wv + bv for all 16 heads, stored [tok, head, 65] with a
           ones-column appended per head (gives softmax sums for free during
           att@V), spilled to DRAM scratch.
  passes p=0..7 (heads 2p, 2p+1):
    KT[128he, 2048tok] = (wk_p.T @ xT) + bk  (transposed layout)
    QT[128he, 1024tq]  = (wq_p.T @ xT[:, :1024]) + bq
    per (head, tq-tile of 512):
      scoresT[tk,tq] = KT_h.T-slices @ QT_h  (K=64 matmuls, one per tk-tile)
      exp on ScalarE straight from PSUM with scale=1/8 (softmax max-subtraction
      skipped: |score/8| <= ~12 so exp is fp32-safe)
      attT[65,tq] += [V_h | 1].T @ expT  (row 64 accumulates the softmax sum)
      normalize: recip(sum) on VectorE, broadcast across partitions on GpSimd
      (partition_broadcast), multiply on VectorE — no PE involvement
  phase 2: out[tok,1024] = [att | 1] @ [wo.T ; bo]  (bias via ones-row matmul)

Scheduling: wkq/kt/qt pools are double-buffered so the Tile scheduler can run
the NEXT pass's K/Q projections as PE filler during the current pass's
attention inner loop (which is otherwise rate-limited by ScalarE exp at ~1us
per 2-ktile group, stalling the PE and dropping its p-state).

All matmuls run in float32r (tf32-like, full PE rate at N>=512).
"""

import numpy as np

import concourse.bass as bass
import concourse.mybir as mybir
import concourse.tile as tile
from concourse import bacc
from concourse.bass_utils import run_bass_kernel_spmd

FP32 = mybir.dt.float32
FP32R = mybir.dt.float32r
AF = mybir.ActivationFunctionType

B, S, D, H, E = 4, 2048, 1024, 16, 64
NCORES = 8
TQ = S // 2  # query tokens per core
SCALE = 1.0 / float(np.sqrt(E))

_CACHE = {}


def build_nc():
    nc = bacc.Bacc("TRN2", target_bir_lowering=False)

    xT = nc.dram_tensor("xT", [D, S], FP32R, kind="ExternalInput")
    wq_t = nc.dram_tensor("wq_t", [D, H * E], FP32R, kind="ExternalInput")
    wk_t = nc.dram_tensor("wk_t", [D, H * E], FP32R, kind="ExternalInput")
    wv_t = nc.dram_tensor("wv_t", [D, H * E], FP32R, kind="ExternalInput")
    wo_t = nc.dram_tensor("wo_t", [D, D], FP32R, kind="ExternalInput")
    bqp = nc.dram_tensor("bqp", [128, 8], FP32, kind="ExternalInput")
    bkp = nc.dram_tensor("bkp", [128, 8], FP32, kind="ExternalInput")
    bv_row = nc.dram_tensor("bv_row", [1, H * E], FP32R, kind="ExternalInput")
    bo_row = nc.dram_tensor("bo_row", [1, D], FP32R, kind="ExternalInput")
    out = nc.dram_tensor("out", [TQ, D], FP32, kind="ExternalOutput")
    # V spill: [tok-tile, tok-in-tile, head, E+1]
    v_spill = nc.dram_tensor("v_spill", [16, 128, H, E + 1], FP32R)

    xT_r = xT.rearrange("(t p) s -> p t s", p=128)  # [128, 8, 2048]
    wq_r = wq_t.rearrange("(t p) m -> p t m", p=128)  # [128, 8, 1024]
    wk_r = wk_t.rearrange("(t p) m -> p t m", p=128)
    wv_r = wv_t.rearrange("(t p) m -> p t m", p=128)
    wo_r = wo_t.rearrange("(t p) m -> p t m", p=128)

    with tile.TileContext(nc) as tc:
        with (
            tc.tile_pool(name="xt", bufs=1) as xt_pool,
            tc.tile_pool(name="wkq", bufs=2) as wkq_pool,
            tc.tile_pool(name="ktp", bufs=2) as kt_pool,
            tc.tile_pool(name="bigw", bufs=2) as bigw_pool,
            tc.tile_pool(name="qt", bufs=2) as qt_pool,
            tc.tile_pool(name="vbuf", bufs=2) as vbuf_pool,
            tc.tile_pool(name="expp", bufs=2) as exp_pool,
            tc.tile_pool(name="attT", bufs=8) as attT_pool,
            tc.tile_pool(name="small", bufs=2) as small_pool,
            tc.tile_pool(name="ones", bufs=1) as ones_pool,
            tc.tile_pool(name="ps_s", bufs=2, space="PSUM") as ps_scores,
            tc.tile_pool(name="ps_a", bufs=2, space="PSUM") as ps_att,
            tc.tile_pool(name="ps_g", bufs=2, space="PSUM") as ps_gen,
        ):
            # ---- persistent tiles ----
            xt_sb = xt_pool.tile([128, 8, S], FP32R, tag="xt")  # 64KB/part
            nc.sync.dma_start(out=xt_sb, in_=xT_r)

            ones_row_f = ones_pool.tile([1, 128], FP32, tag="onesrf")
            nc.vector.memset(ones_row_f, 1.0)
            ones_sb = ones_pool.tile([1, 128], FP32R, tag="ones")
            nc.vector.tensor_copy(out=ones_sb, in_=ones_row_f)
            ones_col_f = ones_pool.tile([128, 8], FP32, tag="onescf")
            nc.vector.memset(ones_col_f, 1.0)
            ones_col = ones_pool.tile([128, 8], FP32R, tag="onescol")
            nc.vector.tensor_copy(out=ones_col, in_=ones_col_f)
            bq_sb = ones_pool.tile([128, 8], FP32, tag="bq")
            bk_sb = ones_pool.tile([128, 8], FP32, tag="bk")
            nc.sync.dma_start(out=bq_sb, in_=bqp[:, :])
            nc.sync.dma_start(out=bk_sb, in_=bkp[:, :])
            bv_sb = ones_pool.tile([1, H * E], FP32R, tag="bv")
            bo_sb = ones_pool.tile([1, D], FP32R, tag="bo")
            nc.sync.dma_start(out=bv_sb, in_=bv_row[:, :])
            nc.sync.dma_start(out=bo_sb, in_=bo_row[:, :])

            attT_tiles = [
                attT_pool.tile([128, TQ], FP32R, tag="attT", name=f"attT{i}")
                for i in range(8)
            ]

            # ---- phase 0: V projection for all heads, spill to DRAM ----
            for nt in range(2):
                wv_sb = bigw_pool.tile([128, 8, 512], FP32R, tag="bigw")
                nc.sync.dma_start(out=wv_sb, in_=wv_r[:, :, nt * 512 : (nt + 1) * 512])
                for tokt in range(16):
                    ps = ps_gen.tile([128, 512], FP32, tag="gen")
                    for k in range(8):
                        nc.tensor.matmul(
                            out=ps,
                            lhsT=xt_sb[:, k, tokt * 128 : (tokt + 1) * 128],
                            rhs=wv_sb[:, k, :],
                            start=(k == 0),
                            stop=False,
                        )
                    nc.tensor.matmul(
                        out=ps,
                        lhsT=ones_sb[:, :128],
                        rhs=bv_sb[:, nt * 512 : (nt + 1) * 512],
                        start=False,
                        stop=True,
                    )
                    vstage = vbuf_pool.tile([128, 8, E + 1], FP32R, tag="vbuf")
                    nc.vector.tensor_copy(
                        out=vstage[:, :, :E],
                        in_=ps.rearrange("p (h e) -> p h e", e=E),
                    )
                    nc.vector.tensor_copy(
                        out=vstage[:, :, E : E + 1], in_=ones_col.unsqueeze(2)
                    )
                    nc.sync.dma_start(
                        out=v_spill[tokt, :, nt * 8 : (nt + 1) * 8, :], in_=vstage
                    )

            # ---- passes: 2 heads each ----
            for p in range(8):
                wk_sb = wkq_pool.tile([128, 8, 128], FP32R, tag="wk")
                wq_sb = wkq_pool.tile([128, 8, 128], FP32R, tag="wq")
                nc.sync.dma_start(out=wk_sb, in_=wk_r[:, :, p * 128 : (p + 1) * 128])
                nc.sync.dma_start(out=wq_sb, in_=wq_r[:, :, p * 128 : (p + 1) * 128])

                kt_sb = kt_pool.tile([128, S], FP32R, tag="kt")
                qt_sb = qt_pool.tile([128, TQ], FP32R, tag="qt")

                for ts in range(4):
                    ps = ps_gen.tile([128, 512], FP32, tag="gen")
                    for k in range(8):
                        nc.tensor.matmul(
                            out=ps,
                            lhsT=wk_sb[:, k, :],
                            rhs=xt_sb[:, k, ts * 512 : (ts + 1) * 512],
                            start=(k == 0),
                            stop=(k == 7),
                        )
                    nc.vector.tensor_scalar_add(
                        out=kt_sb[:, ts * 512 : (ts + 1) * 512],
                        in0=ps,
                        scalar1=bk_sb[:, p : p + 1],
                    )
                for qs in range(2):
                    ps = ps_gen.tile([128, 512], FP32, tag="gen")
                    for k in range(8):
                        nc.tensor.matmul(
                            out=ps,
                            lhsT=wq_sb[:, k, :],
                            rhs=xt_sb[:, k, qs * 512 : (qs + 1) * 512],
                            start=(k == 0),
                            stop=(k == 7),
                        )
                    nc.vector.tensor_scalar_add(
                        out=qt_sb[:, qs * 512 : (qs + 1) * 512],
                        in0=ps,
                        scalar1=bq_sb[:, p : p + 1],
                    )

                for hh in range(2):
                    base = hh * 64
                    h = 2 * p + hh
                    vh_sb = vbuf_pool.tile([128, 16, E + 1], FP32R, tag="vbuf")
                    nc.sync.dma_start(
                        out=vh_sb, in_=v_spill[:, :, h, :].transpose([1, 0, 2])
                    )
                    for tqt in range(2):
                        att_ps = ps_att.tile([E + 1, 512], FP32, tag="att")
                        for g in range(8):
                            ps_s = ps_scores.tile([128, 2, 512], FP32, tag="sc")
                            for j in range(2):
                                t = g * 2 + j
                                nc.tensor.matmul(
                                    out=ps_s[:, j, :],
                                    lhsT=kt_sb[
                                        base : base + 64, t * 128 : (t + 1) * 128
                                    ],
                                    rhs=qt_sb[
                                        base : base + 64, tqt * 512 : (tqt + 1) * 512
                                    ],
                                    start=True,
                                    stop=True,
                                )
                            exp_t = exp_pool.tile([128, 2, 512], FP32R, tag="exp")
                            nc.scalar.activation(
                                out=exp_t, in_=ps_s, func=AF.Exp, scale=SCALE
                            )
                            for j in range(2):
                                t = g * 2 + j
                                nc.tensor.matmul(
                                    out=att_ps,
                                    lhsT=vh_sb[:, t, :],
                                    rhs=exp_t[:, j, :],
                                    start=(t == 0),
                                    stop=(t == 15),
                                )
                        recip_r = small_pool.tile([1, 512], FP32, tag="recr", bufs=2)
                        with nc.allow_low_precision(reason="softmax recip"):
                            nc.vector.reciprocal(out=recip_r, in_=att_ps[E : E + 1, :])
                        rb_sb = small_pool.tile([64, 512], FP32, tag="rbb", bufs=2)
                        nc.gpsimd.partition_broadcast(rb_sb, recip_r)
                        nc.vector.tensor_mul(
                            out=attT_tiles[p][
                                base : base + 64, tqt * 512 : (tqt + 1) * 512
                            ],
                            in0=att_ps[:E, :],
                            in1=rb_sb,
                        )

            # ---- phase 2: output projection ----
            wo_sb = [
                bigw_pool.tile([128, 8, 512], FP32R, tag="bigw", name=f"wo{i}")
                for i in range(2)
            ]
            for nt in range(2):
                nc.sync.dma_start(
                    out=wo_sb[nt], in_=wo_r[:, :, nt * 512 : (nt + 1) * 512]
                )
            for tokt in range(8):
                for nt in range(2):
                    ps = ps_gen.tile([128, 512], FP32, tag="gen")
                    for t in range(8):
                        nc.tensor.matmul(
                            out=ps,
                            lhsT=attT_tiles[t][:, tokt * 128 : (tokt + 1) * 128],
                            rhs=wo_sb[nt][:, t, :],
                            start=(t == 0),
                            stop=False,
                        )
                    nc.tensor.matmul(
                        out=ps,
                        lhsT=ones_sb[:, :128],
                        rhs=bo_sb[:, nt * 512 : (nt + 1) * 512],
                        start=False,
                        stop=True,
                    )
                    ostg = small_pool.tile([128, 512], FP32, tag="stg", bufs=2)
                    nc.vector.tensor_copy(out=ostg, in_=ps)
                    nc.sync.dma_start(
                        out=out[tokt * 128 : (tokt + 1) * 128, nt * 512 : (nt + 1) * 512],
                        in_=ostg,
                    )

    nc.compile()
    return nc


def kernel(x, wq, bq, wk, bk, wv, bv, wo, bo, trace=False):
    x = np.asarray(x, dtype=np.float32)
    wq = np.asarray(wq, dtype=np.float32)
    bq = np.asarray(bq, dtype=np.float32)
    wk = np.asarray(wk, dtype=np.float32)
    bk = np.asarray(bk, dtype=np.float32)
    wv = np.asarray(wv, dtype=np.float32)
    bv = np.asarray(bv, dtype=np.float32)
    wo = np.asarray(wo, dtype=np.float32)
    bo = np.asarray(bo, dtype=np.float32)

    if "nc" not in _CACHE:
        _CACHE["nc"] = build_nc()
    nc = _CACHE["nc"]

    wq_t = np.ascontiguousarray(wq.transpose(1, 0, 2).reshape(D, H * E))
    wk_t = np.ascontiguousarray(wk.transpose(1, 0, 2).reshape(D, H * E))
    wv_t = np.ascontiguousarray(wv.transpose(1, 0, 2).reshape(D, H * E))
    wo_t = np.ascontiguousarray(wo.T)
    bqp = np.ascontiguousarray(bq.reshape(H * E).reshape(8, 128).T)
    bkp = np.ascontiguousarray(bk.reshape(H * E).reshape(8, 128).T)
    bv_row = np.ascontiguousarray(bv.reshape(1, H * E))
    bo_row = np.ascontiguousarray(bo.reshape(1, D))

    shared = {
        "wq_t": wq_t,
        "wk_t": wk_t,
        "wv_t": wv_t,
        "wo_t": wo_t,
        "bqp": bqp,
        "bkp": bkp,
        "bv_row": bv_row,
        "bo_row": bo_row,
    }
    in_maps = []
    for c in range(NCORES):
        b, qh = c // 2, c % 2
        xT_b = x[b].T  # [D, S]
        if qh == 0:
            xT_c = np.ascontiguousarray(xT_b)
        else:
            # my query tokens first; K/V token order is irrelevant as long as
            # K and V agree (softmax + weighted sum are permutation-invariant)
            xT_c = np.ascontiguousarray(
                np.concatenate([xT_b[:, TQ:], xT_b[:, :TQ]], axis=1)
            )
        m = dict(shared)
        m["xT"] = xT_c
        in_maps.append(m)

    res = run_bass_kernel_spmd(nc, in_maps, list(range(NCORES)), trace=trace)

    out = np.empty((B, S, D), dtype=np.float32)
    for c in range(NCORES):
        b, qh = c // 2, c % 2
        out[b, qh * TQ : (qh + 1) * TQ, :] = res.results[c]["out"]
    if trace:
        return out, res
    return out
